# revision 6
# baseline (speedup 1.0000x reference)
"""Single-launch Trainium2 kernel for nn_PolyAttn (B=4, N=2048, D=H=1024).

Math: the reference's score normalization a/|a| with a = (.)^4 >= 0 makes the
attention matrix exactly all-ones, so

    out[b, n, :] = r[b, :],  r = xs @ W_v @ w_o,  xs[b] = sum_n x[b, n, :].

The matmul chain is reassociated as r = xs @ (W_v @ w_o) so the cross-core
reduction moves to the very end, where it is a pure gather: with the
contraction dim hidden-sharded (core i owns channels 128i..128(i+1)),

    r = sum_i  xs[:, chunk_i] @ (W_v[chunk_i, :] @ w_o)
      = sum_i  xs_chunk_i @ M_i,

and each core computes its full-width partial independently in ONE launch.
Per core, everything overlaps the 4.5 MiB fp16 input stream (x chunk 2 MiB +
full w_o 2 MiB + wvT 0.25 MiB, pieces interleaved across the two HWDGE
queues so both compute engines stay fed):

  - DVE folds the 8 x pieces with a running fp16 add chain (2x packed mode),
    then one free-dim reduce -> xsT_chunk [128, 4] (fp32), cast to fp16.
  - PE accumulates M_i = Wv[chunk_i, :] @ w_o from the fp16 weight tiles as
    two 512-col PSUM groups x 8 K-blocks, paced by the wo block arrivals.
  - ACT copies M_i from PSUM to SBUF as fp16 (two halves, each right after
    its accumulation group stops).
  - PE finale: rT_i[m, 4j+b] = (xs_chunk @ M_i)[b, 128j+m] via 8 tiny
    matmuls (stationary M16 block, moving xsT); DVE copies PSUM->SBUF and
    the [128, 32] fp32 partial is DMA'd out.

Host does only gather/unshard work: sum the 8 partials, rearrange the
transposed layout, broadcast over the sequence dim (attention is all-ones).
"""

import numpy as np

import concourse.bacc as bacc
import concourse.mybir as mybir
from concourse.bass_utils import run_bass_kernel_spmd

NCORES = 8
B, N, D, H = 4, 2048, 1024, 1024
NK = 16         # x stream pieces per core (2 pieces per DMA)
NP = N // NK    # 128 seq positions per piece
F16 = mybir.dt.float16
F32 = mybir.dt.float32
AX = mybir.AxisListType
ALU = mybir.AluOpType

_BUILT = {}


def _build():
    nc = bacc.Bacc("TRN2", target_bir_lowering=False, debug=False,
                   num_devices=NCORES)
    xh = nc.dram_tensor("xh", [128, NK, B, NP], F16, kind="ExternalInput")
    wvT = nc.dram_tensor("wvT", [128, 8, 128], F16, kind="ExternalInput")
    wo = nc.dram_tensor("wo", [128, 8, 1024], F16, kind="ExternalInput")
    rp = nc.dram_tensor("rpartT", [128, 32], F32, kind="ExternalOutput")

    xh_sb = nc.alloc_sbuf_tensor("xh_sb", [128, NK, B, NP], F16)
    wvT_sb = nc.alloc_sbuf_tensor("wvT_sb", [128, 8, 128], F16)
    wo_sb = nc.alloc_sbuf_tensor("wo_sb", [128, 8, 1024], F16)
    acc = nc.alloc_sbuf_tensor("acc", [128, B, NP], F16)
    xs16 = nc.alloc_sbuf_tensor("xs16", [128, B], F16)
    xsA16 = nc.alloc_sbuf_tensor("xsA16", [128, B], F16)
    xsC16 = nc.alloc_sbuf_tensor("xsC16", [128, B], F16)
    xsD16 = nc.alloc_sbuf_tensor("xsD16", [128, B], F16)
    xsE16 = nc.alloc_sbuf_tensor("xsE16", [128, B], F16)
    m16 = nc.alloc_sbuf_tensor("m16", [128, 1024], F16)
    ro = nc.alloc_sbuf_tensor("ro", [128, 32], F32)

    pm = nc.alloc_psum_tensor("pm", [128, 1024], F32)
    prT = nc.alloc_psum_tensor("prT", [128, 32], F32)
    wp = nc.alloc_psum_tensor("wp", [128, 4], F32)

    # one semaphore per DMA (HWDGE completions within a queue are unordered)
    x_s = [nc.alloc_semaphore(f"x_s{k}") for k in range(NK // 2)]
    wo_s = [nc.alloc_semaphore(f"wo_s{a}") for a in range(8)]
    wvT_s = nc.alloc_semaphore("wvT_s")
    out_s = nc.alloc_semaphore("out_s")
    v_s = nc.alloc_semaphore("v_s")
    a_s = nc.alloc_semaphore("a_s")
    pe_s = nc.alloc_semaphore("pe_s")

    with nc.Block(no_gpsimd_drain=True) as block:

        # Stream order: even x pieces + wo pairs 01/23 on sync, odd x pieces
        # + wo pairs 45/67 on scalar, so piece pairs (2k, 2k+1) arrive
        # together and wo pairs land spread through the stream.  wvT (tiny,
        # needed by every M matmul) goes first.
        # Weights lead each queue so the M chain closes early; the last
        # three x DMAs fold through direct reduces (no serial add chain),
        # and both queues end with a small x DMA so the end receipts overlap
        @block.sync
        def _(sync):
            sync.dma_start(wvT_sb[:], wvT[:]).then_inc(wvT_s, 16)
            sync.dma_start(wo_sb[:, 0:2, :], wo[:, 0:2, :]).then_inc(wo_s[0], 16)
            sync.dma_start(xh_sb[:, 0:4, :, :], xh[:, 0:4, :, :]).then_inc(x_s[0], 16)
            sync.dma_start(wo_sb[:, 2:4, :], wo[:, 2:4, :]).then_inc(wo_s[2], 16)
            sync.dma_start(xh_sb[:, 12:14, :, :], xh[:, 12:14, :, :]).then_inc(x_s[6], 16)
            sync.wait_ge(v_s, 15)
            sync.dma_start(rp[:], ro[:]).then_inc(out_s, 16)
            # no out_s wait: the teardown's queue drain flushes the store

        @block.scalar
        def _(scalar):
            scalar.dma_start(wo_sb[:, 4:6, :], wo[:, 4:6, :]).then_inc(wo_s[4], 16)
            scalar.dma_start(xh_sb[:, 4:8, :, :], xh[:, 4:8, :, :]).then_inc(x_s[1], 16)
            scalar.dma_start(wo_sb[:, 6:8, :], wo[:, 6:8, :]).then_inc(wo_s[6], 16)
            scalar.dma_start(xh_sb[:, 8:12, :, :], xh[:, 8:12, :, :]).then_inc(x_s[2], 16)
            scalar.dma_start(xh_sb[:, 14:16, :, :], xh[:, 14:16, :, :]).then_inc(x_s[7], 16)
            # M16 = fp16(M), PSUM -> SBUF on the ACT engine; each half right
            # after its accumulation group stops (a-last matmuls: pe_s 15/16)
            scalar.wait_ge(pe_s, 15)
            scalar.copy(m16[:, :512], pm[:, :512]).then_inc(a_s, 1)
            scalar.wait_ge(pe_s, 16)
            scalar.copy(m16[:, 512:], pm[:, 512:]).then_inc(a_s, 1)

        @block.tensor
        def _(tensor):
            # early warm-up on wvT (first arrival) so the PE clock ramps
            # before the M phase
            tensor.wait_ge(wvT_s, 16)
            for _ in range(4):
                tensor.matmul(wp[:], wvT_sb[:, 0, :], wvT_sb[:, 0, :4],
                              start=True, stop=True)
            # M_i = Wv_chunk @ w_o: 8 K-blocks x two 512-col PSUM groups,
            # ordered/paced by the wo block arrivals (overlaps the x stream);
            # junk matmuls between pairs keep the activity monitor hot
            a_order = [4, 5, 0, 1, 6, 7, 2, 3]
            for n, a in enumerate(a_order):
                tensor.wait_ge(wo_s[a // 2 * 2], 16)
                for h in range(2):
                    tensor.matmul(pm[:, 512 * h: 512 * (h + 1)],
                                  wvT_sb[:, a, :],
                                  wo_sb[:, a, 512 * h: 512 * (h + 1)],
                                  start=(n == 0), stop=(n == 7)) \
                        .then_inc(pe_s, 1)
                tensor.matmul(wp[:], wvT_sb[:, 0, :], wvT_sb[:, 0, :4],
                              start=True, stop=True)
            # finale: rT_i[:, 4j:4j+4] = M16_block_j^T @ xsT
            tensor.wait_ge(v_s, 14)
            tensor.wait_ge(a_s, 2)
            for j in range(8):
                tensor.matmul(prT[:, 4 * j: 4 * (j + 1)],
                              m16[:, 128 * j: 128 * (j + 1)], xs16[:],
                              start=True, stop=True).then_inc(pe_s, 1)

        @block.vector
        def _(vector):
            # pieces 0-7 (the two early 512K DMAs) fold through a running
            # fp16 add chain; the three late DMAs fold through independent
            # strided reduces so no serial chain trails the last arrivals
            vector.wait_ge(x_s[0], 16)
            vector.tensor_add(acc[:], xh_sb[:, 0, :, :], xh_sb[:, 1, :, :]) \
                .then_inc(v_s, 1)
            v = 1
            for p in range(2, 8):
                vector.wait_ge(x_s[0 if p < 4 else 1], 16)
                vector.wait_ge(v_s, v)
                vector.tensor_add(acc[:], acc[:], xh_sb[:, p, :, :]) \
                    .then_inc(v_s, 1)
                v += 1  # -> 7
            with nc.allow_low_precision(reason="xs fp16 feeds fp16 matmul"):
                vector.wait_ge(v_s, 7)
                vector.tensor_reduce(xsA16[:], acc[:], axis=AX.X, op=ALU.add) \
                    .then_inc(v_s, 1)  # -> 8
                # late DMAs: per-batch sums over (piece, n') via strided views
                vector.wait_ge(x_s[2], 16)
                vector.tensor_reduce(
                    xsC16[:], xh_sb[:, 8:12, :, :].transpose([0, 2, 1, 3]),
                    axis=AX.XY, op=ALU.add).then_inc(v_s, 1)  # -> 9
                vector.wait_ge(x_s[6], 16)
                vector.tensor_reduce(
                    xsD16[:], xh_sb[:, 12:14, :, :].transpose([0, 2, 1, 3]),
                    axis=AX.XY, op=ALU.add).then_inc(v_s, 1)  # -> 10
                vector.wait_ge(x_s[7], 16)
                vector.tensor_reduce(
                    xsE16[:], xh_sb[:, 14:16, :, :].transpose([0, 2, 1, 3]),
                    axis=AX.XY, op=ALU.add).then_inc(v_s, 1)  # -> 11
            vector.wait_ge(v_s, 9)
            vector.tensor_add(xs16[:], xsA16[:], xsC16[:]).then_inc(v_s, 1)
            vector.wait_ge(v_s, 12)
            vector.tensor_add(xs16[:], xs16[:], xsD16[:]).then_inc(v_s, 1)
            vector.wait_ge(v_s, 13)
            vector.tensor_add(xs16[:], xs16[:], xsE16[:]).then_inc(v_s, 1)  # 14
            vector.wait_ge(pe_s, 24)
            vector.tensor_copy(ro[:], prT[:]).then_inc(v_s, 1)  # -> 15

    nc.compile()
    return nc


def _get(name, builder):
    if name not in _BUILT:
        _BUILT[name] = builder()
    return _BUILT[name]


def kernel(x, w_qkv, w_o, alpha):
    x = np.asarray(x, dtype=np.float32)
    w_qkv = np.asarray(w_qkv, dtype=np.float32)
    w_o = np.asarray(w_o, dtype=np.float32)
    core_ids = list(range(NCORES))

    nc = _get("m", _build)
    # xh[c, k, b, n'] = x[b, NP*k + n', c], contiguous fp16 per core slice
    xt = x.reshape(B, NK, NP, D).transpose(3, 1, 0, 2).astype(np.float16)
    # wvt[k, a, i, m] = wv[128i + m, 128a + k]  (lhsT blocks for M)
    wv = w_qkv[:, 2 * H: 3 * H]
    wvt = wv.reshape(8, 128, 8, 128).transpose(3, 2, 0, 1).astype(np.float16)
    # woh[k, a, n] = w_o[128a + k, n]  (rhs blocks for M)
    woh = np.ascontiguousarray(
        w_o.reshape(8, 128, 1024).transpose(1, 0, 2).astype(np.float16))
    in_maps = []
    for i in range(NCORES):
        in_maps.append({
            "xh": xt[128 * i: 128 * (i + 1)],
            "wvT": np.ascontiguousarray(wvt[:, :, i, :]),
            "wo": woh,
        })
    res = run_bass_kernel_spmd(nc, in_maps, core_ids)

    # gather: sum the 8 transposed partials, rearrange [m, j, b] -> [b, 128j+m]
    rT = np.sum([r["rpartT"] for r in res.results], axis=0)  # [128, 32]
    r = rT.reshape(128, 8, B).transpose(2, 1, 0).reshape(B, D)

    out = np.broadcast_to(r[:, None, :], (B, N, D))
    return np.ascontiguousarray(out)


# revision 8
# speedup vs baseline: 1.0059x; 1.0059x over previous
"""Single-launch Trainium2 kernel for nn_PolyAttn (B=4, N=2048, D=H=1024).

Math: the reference's score normalization a/|a| with a = (.)^4 >= 0 makes the
attention matrix exactly all-ones, so

    out[b, n, :] = r[b, :],  r = xs @ W_v @ w_o,  xs[b] = sum_n x[b, n, :].

The matmul chain is reassociated as r = xs @ (W_v @ w_o) so the cross-core
reduction moves to the very end, where it is a pure gather: with the
contraction dim hidden-sharded (core i owns channels 128i..128(i+1)),

    r = sum_i  xs[:, chunk_i] @ (W_v[chunk_i, :] @ w_o)
      = sum_i  xs_chunk_i @ M_i,

and each core computes its full-width partial independently in ONE launch.
Per core, everything overlaps the 4.5 MiB fp16 input stream (x chunk 2 MiB +
full w_o 2 MiB + wvT 0.25 MiB, pieces interleaved across the two HWDGE
queues so both compute engines stay fed):

  - DVE folds the 8 x pieces with a running fp16 add chain (2x packed mode),
    then one free-dim reduce -> xsT_chunk [128, 4] (fp32), cast to fp16.
  - PE accumulates M_i = Wv[chunk_i, :] @ w_o from the fp16 weight tiles as
    two 512-col PSUM groups x 8 K-blocks, paced by the wo block arrivals.
  - ACT copies M_i from PSUM to SBUF as fp16 (two halves, each right after
    its accumulation group stops).
  - PE finale: rT_i[m, 4j+b] = (xs_chunk @ M_i)[b, 128j+m] via 8 tiny
    matmuls (stationary M16 block, moving xsT); DVE copies PSUM->SBUF and
    the [128, 32] fp32 partial is DMA'd out.

Host does only gather/unshard work: sum the 8 partials, rearrange the
transposed layout, broadcast over the sequence dim (attention is all-ones).
"""

import numpy as np

import concourse.bacc as bacc
import concourse.mybir as mybir
from concourse.bass_utils import run_bass_kernel_spmd

NCORES = 8
B, N, D, H = 4, 2048, 1024, 1024
NK = 16         # x stream pieces per core (2 pieces per DMA)
NP = N // NK    # 128 seq positions per piece
F16 = mybir.dt.float16
F32 = mybir.dt.float32
AX = mybir.AxisListType
ALU = mybir.AluOpType

_BUILT = {}


def _build():
    nc = bacc.Bacc("TRN2", target_bir_lowering=False, debug=False,
                   num_devices=NCORES)
    xh = nc.dram_tensor("xh", [128, NK, B, NP], F16, kind="ExternalInput")
    wo = nc.dram_tensor("wo", [128, 9, 1024], F16, kind="ExternalInput")
    rp = nc.dram_tensor("rpartT", [128, 32], F32, kind="ExternalOutput")

    xh_sb = nc.alloc_sbuf_tensor("xh_sb", [128, NK, B, NP], F16)
    wo_sb = nc.alloc_sbuf_tensor("wo_sb", [128, 9, 1024], F16)
    acc = nc.alloc_sbuf_tensor("acc", [128, B, NP], F16)
    xsf = nc.alloc_sbuf_tensor("xsf", [128, B], F32)
    xs16 = nc.alloc_sbuf_tensor("xs16", [128, B], F16)
    m16 = nc.alloc_sbuf_tensor("m16", [128, 1024], F16)
    ro = nc.alloc_sbuf_tensor("ro", [128, 32], F32)

    pm = nc.alloc_psum_tensor("pm", [128, 1024], F32)
    prT = nc.alloc_psum_tensor("prT", [128, 32], F32)
    wp = nc.alloc_psum_tensor("wp", [128, 4], F32)

    # one semaphore per DMA (HWDGE completions within a queue are unordered)
    x_s = [nc.alloc_semaphore(f"x_s{k}") for k in range(NK // 2)]
    wo_s = [nc.alloc_semaphore(f"wo_s{a}") for a in range(8)]
    out_s = nc.alloc_semaphore("out_s")
    v_s = nc.alloc_semaphore("v_s")
    a_s = nc.alloc_semaphore("a_s")
    pe_s = nc.alloc_semaphore("pe_s")

    with nc.Block(no_gpsimd_drain=True) as block:

        # Stream order: even x pieces + wo pairs 01/23 on sync, odd x pieces
        # + wo pairs 45/67 on scalar, so piece pairs (2k, 2k+1) arrive
        # together and wo pairs land spread through the stream.  wvT (tiny,
        # needed by every M matmul) goes first.
        # 6 x DMAs (3x512K early, 2x256K last) and 4 wo DMAs spread so both
        # queues end with a small x DMA and the two end receipts overlap
        @block.sync
        def _(sync):
            sync.dma_start(wo_sb[:, 0:3, :], wo[:, 0:3, :]).then_inc(wo_s[0], 16)
            sync.dma_start(xh_sb[:, 0:4, :, :], xh[:, 0:4, :, :]).then_inc(x_s[0], 16)
            sync.dma_start(xh_sb[:, 8:12, :, :], xh[:, 8:12, :, :]).then_inc(x_s[2], 16)
            sync.dma_start(xh_sb[:, 12:14, :, :], xh[:, 12:14, :, :]).then_inc(x_s[6], 16)
            sync.wait_ge(v_s, 17)
            sync.dma_start(rp[:], ro[:]).then_inc(out_s, 16)
            # no out_s wait: the teardown's queue drain flushes the store

        @block.scalar
        def _(scalar):
            scalar.dma_start(xh_sb[:, 4:8, :, :], xh[:, 4:8, :, :]).then_inc(x_s[1], 16)
            scalar.dma_start(wo_sb[:, 5:7, :], wo[:, 5:7, :]).then_inc(wo_s[4], 16)
            scalar.dma_start(wo_sb[:, 3:5, :], wo[:, 3:5, :]).then_inc(wo_s[2], 16)
            scalar.dma_start(wo_sb[:, 7:9, :], wo[:, 7:9, :]).then_inc(wo_s[6], 16)
            scalar.dma_start(xh_sb[:, 14:16, :, :], xh[:, 14:16, :, :]).then_inc(x_s[7], 16)
            # M16 = fp16(M), PSUM -> SBUF on the ACT engine; each half right
            # after its accumulation group stops (a-last matmuls: pe_s 15/16)
            scalar.wait_ge(pe_s, 15)
            scalar.copy(m16[:, :512], pm[:, :512]).then_inc(a_s, 1)
            scalar.wait_ge(pe_s, 16)
            scalar.copy(m16[:, 512:], pm[:, 512:]).then_inc(a_s, 1)

        @block.tensor
        def _(tensor):
            # M_i = Wv_chunk @ w_o: 8 K-blocks x two 512-col PSUM groups,
            # ordered/paced by the wo block arrivals (overlaps the x stream);
            # junk matmuls between pairs keep the activity monitor hot.
            # wvT rides block 0 of the wo tensor; data blocks are 1+a.
            a_order = [0, 1, 4, 5, 2, 3, 6, 7]
            for n, a in enumerate(a_order):
                tensor.wait_ge(wo_s[a // 2 * 2], 16)
                for h in range(2):
                    tensor.matmul(pm[:, 512 * h: 512 * (h + 1)],
                                  wo_sb[:, 0, 128 * a: 128 * (a + 1)],
                                  wo_sb[:, 1 + a, 512 * h: 512 * (h + 1)],
                                  start=(n == 0), stop=(n == 7)) \
                        .then_inc(pe_s, 1)
                tensor.matmul(wp[:], wo_sb[:, 0, :128], wo_sb[:, 0, :4],
                              start=True, stop=True)
            # finale: rT_i[:, 4j:4j+4] = M16_block_j^T @ xsT
            tensor.wait_ge(v_s, 16)
            tensor.wait_ge(a_s, 2)
            for j in range(8):
                tensor.matmul(prT[:, 4 * j: 4 * (j + 1)],
                              m16[:, 128 * j: 128 * (j + 1)], xs16[:],
                              start=True, stop=True).then_inc(pe_s, 1)

        @block.vector
        def _(vector):
            # running fp16 fold of the 16 x pieces in DMA-arrival order
            # (tensor_tensor add runs the 2x packed mode; each add waits its
            # DMA's sem + the previous add)
            groups = [(0, [0, 1, 2, 3]), (1, [4, 5, 6, 7]),
                      (2, [8, 9, 10, 11]), (6, [12, 13]), (7, [14, 15])]
            vector.wait_ge(x_s[0], 16)
            vector.tensor_add(acc[:], xh_sb[:, 0, :, :], xh_sb[:, 1, :, :]) \
                .then_inc(v_s, 1)
            v = 1
            for j, pieces in groups:
                for p in pieces:
                    if p < 2:
                        continue
                    vector.wait_ge(x_s[j], 16)
                    vector.wait_ge(v_s, v)
                    vector.tensor_add(acc[:], acc[:], xh_sb[:, p, :, :]) \
                        .then_inc(v_s, 1)
                    v += 1
            # free-dim fold straight to fp16  (v == 15 here)
            vector.wait_ge(v_s, 15)
            with nc.allow_low_precision(reason="xs fp16 feeds fp16 matmul"):
                vector.tensor_reduce(xs16[:], acc[:], axis=AX.X, op=ALU.add) \
                    .then_inc(v_s, 1)  # -> 16
            vector.wait_ge(pe_s, 24)
            vector.tensor_copy(ro[:], prT[:]).then_inc(v_s, 1)  # -> 17

    nc.compile()
    return nc


def _get(name, builder):
    if name not in _BUILT:
        _BUILT[name] = builder()
    return _BUILT[name]


def kernel(x, w_qkv, w_o, alpha):
    x = np.asarray(x, dtype=np.float32)
    w_qkv = np.asarray(w_qkv, dtype=np.float32)
    w_o = np.asarray(w_o, dtype=np.float32)
    core_ids = list(range(NCORES))

    nc = _get("m", _build)
    # xh[c, k, b, n'] = x[b, NP*k + n', c], contiguous fp16 per core slice
    xt = x.reshape(B, NK, NP, D).transpose(3, 1, 0, 2).astype(np.float16)
    # wvt[k, a, i, m] = wv[128i + m, 128a + k]  (lhsT blocks for M)
    wv = w_qkv[:, 2 * H: 3 * H]
    wvt = wv.reshape(8, 128, 8, 128).transpose(3, 2, 0, 1).astype(np.float16)
    # woh[k, a, n] = w_o[128a + k, n]  (rhs blocks for M)
    woh = np.ascontiguousarray(
        w_o.reshape(8, 128, 1024).transpose(1, 0, 2).astype(np.float16))
    in_maps = []
    for i in range(NCORES):
        wv0 = wvt[:, :, i, :].reshape(128, 1, 1024)
        in_maps.append({
            "xh": xt[128 * i: 128 * (i + 1)],
            "wo": np.ascontiguousarray(
                np.concatenate([wv0, woh], axis=1)),
        })
    res = run_bass_kernel_spmd(nc, in_maps, core_ids)

    # gather: sum the 8 transposed partials, rearrange [m, j, b] -> [b, 128j+m]
    rT = np.sum([r["rpartT"] for r in res.results], axis=0)  # [128, 32]
    r = rT.reshape(128, 8, B).transpose(2, 1, 0).reshape(B, D)

    out = np.broadcast_to(r[:, None, :], (B, N, D))
    return np.ascontiguousarray(out)
